# revision 52
# baseline (speedup 1.0000x reference)
"""Causal single-head attention on 8 Trainium2 NeuronCores (Bass/Tile).

Problem: X[4,4096,512] fp32, Wq/Wk/Wv[512,64] fp32.
  Q=XWq, K=XWk, V=XWv ; Z = softmax(mask(QK^T)/8) V    -> [4,4096,64]

Sharding: 2 cores per batch, fully uniform SPMD program.
  - Keys/values are split by PARITY of 128-row key blocks: core A of a pair
    owns even key blocks, core B odd ones.  Each core's X^T input is
    ROTATED left by 128*parity columns by the host, which makes "my key
    blocks" sit at even 128-col positions for BOTH cores -- so one
    instruction stream with static addresses serves both.
  - Each core computes, for every query tile, partial attention over its
    own half of the keys with un-normalized softmax (no max subtraction --
    logits here are ~N(0, 0.2^2) so exp cannot overflow):
        numerator   N_c = sum_k exp(s)*V,   denominator D_c = sum_k exp(s)
    The host combines  Z = (N_A + N_B) / (D_A + D_B)  exactly.  The
    rotation wraps one query block on core B (tile 7); the host simply
    uses A-only partials for those 128 queries (A covers them fully).
  - Denominators come for free as column 64 of V_ext = [V | 1] in the
    P^T @ V_ext matmul.
  - Causality at 128-block granularity is structural (k-block count grows
    with the query tile); diagonal blocks are fixed by multiplying exp(S)
    by a static triangular mask (rotation makes the needed mask content
    identical on both cores).

On-chip dataflow:
  - scores are computed transposed  S^T[k,q] = K^T-block-stationary @ Q^T
    (bf16) so P^T = exp(S^T) feeds the PV matmul with no on-chip
    transpose.  Q^T and K^T are doubled across the partition dim ([W|W]
    weights) so score matmuls (contraction=64) run 2x packed in the PE
    array via row groups (partitions 0-63 / 64-127).
  - P^T and V_ext are stored fp8e4 and the PV matmuls run
    perf_mode=DoubleRow: each matmul contracts a PAIR of 128-key blocks
    (virtual 256-deep PE array) in one 512-column stream -- halving PE
    time on the dominant matmul category.  The FIRST key-block pair of
    each core stays bf16 (plain matmuls): early queries average over few
    keys, so fp8 noise on V does not cancel there (rel err 3e-2 -> 3e-3).
  - V is produced in natural [k,64] layout by making the X^T chunk the
    stationary operand; K projection reads even 128-col blocks of X^T via
    a strided access pattern, split in half-chains scheduled just-in-time
    against the X DMA stream.
  - X^T tokens 512+ ship as fp8e4 (mixed bf16-weight x fp8-X matmuls):
    projection noise only perturbs softmax weights / well-averaged V
    contributions, halving 3.5 MB of the DMA stream at no accuracy cost.
    Tokens 0:512 stay bf16 (they feed the bf16 V blocks).  All weights
    ship as ONE host-packed contiguous tensor (2.5 KB per partition) so
    the first DMA lands fast.  Z ships bf16 (halves the output tail).
  - A two-stage memset-fed warmup matmul block keeps the PE continuously
    busy from the preamble until the first input DMA semaphore fires:
    >=4us of it guarantees the HAM clock-gate release lands mid-warmup
    (one idle gap during the ramp re-throttles the clock to 1.2 GHz for
    ~the whole first third); a short second stage gated on the first
    DMA's semaphore absorbs run-to-run DMA completion jitter.
"""

import numpy as np
import ml_dtypes

import concourse.bacc as bacc
import concourse.bass as bass
import concourse.mybir as mybir
import concourse.tile as tile

B, S, DIN, E = 4, 4096, 512, 64
PB = 128            # partition / key block
QT = 512            # query tile width
NQT = S // QT       # 8 query tiles
NKB = S // PB       # 32 key blocks per batch
HKB = NKB // 2      # 16 packed key blocks per core
SH = S // 2         # 2048 packed keys per core
NCORES = 8
SCALE = 1.0 / np.sqrt(E)
GJ = 2              # k-blocks per exp group (= one DoubleRow PV pair)
VW = 80             # padded V_ext block width (Ko step must be %16==0)

BF16 = ml_dtypes.bfloat16
NPF8 = ml_dtypes.float8_e4m3
BF = mybir.dt.bfloat16
F8 = mybir.dt.float8e4
F32 = mybir.dt.float32
DR = mybir.MatmulPerfMode.DoubleRow

_CACHE = {}


def _build():
    nc = bacc.Bacc("TRN2", target_bir_lowering=False, debug=False,
                   enable_asserts=False, num_devices=NCORES)

    xtf_h = nc.dram_tensor("xtf", [DIN, QT], BF, kind="ExternalInput")
    xt8_h = nc.dram_tensor("xt8", [DIN, S - QT], F8, kind="ExternalInput")
    wpk_h = nc.dram_tensor("wpk", [PB, 1280], BF, kind="ExternalInput")
    msk_h = nc.dram_tensor("msk", [PB, QT], F8, kind="ExternalInput")
    mskb_h = nc.dram_tensor("mskb", [PB, QT], BF, kind="ExternalInput")
    zt_h = nc.dram_tensor("zt", [E + 1, S], BF, kind="ExternalOutput")

    xtf_r = xtf_h.ap().rearrange("(c p) s -> p c s", p=PB)
    xt8_r = xt8_h.ap().rearrange("(c p) s -> p c s", p=PB)
    zt = zt_h.ap()

    with tile.TileContext(nc) as tc:
        with (
            tc.tile_pool(name="big", bufs=1) as big,
            tc.tile_pool(name="pt", bufs=10) as ptp,
            tc.tile_pool(name="ptb", bufs=5) as ptbp,
            tc.tile_pool(name="zsb", bufs=2) as zsbp,
            tc.tile_pool(name="ppsum", bufs=3, space="PSUM") as pp,
            tc.tile_pool(name="spsum", bufs=2, space="PSUM") as sp,
            tc.tile_pool(name="zpsum", bufs=1, space="PSUM") as zp,
        ):
            # ---- persistent SBUF buffers ----
            # X^T tokens 0:512 in bf16 (early-query V/P precision);
            # tokens 512: in fp8 (softmax noise cancels -- halves DMA)
            xtf_sb = big.tile([PB, 4, QT], BF, tag="xtf")
            xt8_sb = big.tile([PB, 4, S - QT], F8, tag="xt8")
            wpk_sb = big.tile([PB, 1280], BF, tag="wpk")
            msk_sb = big.tile([PB, QT], F8, tag="msk")
            mskb_sb = big.tile([PB, QT], BF, tag="mskb")
            qt2 = big.tile([PB, S], BF, tag="qt2")      # doubled Q^T (rot)
            kt2 = big.tile([PB, SH], BF, tag="kt2")     # doubled K^T (packed)
            vext = big.tile([PB, HKB, VW], F8, tag="vext")
            vextb = big.tile([PB, 2, E + 1], BF, tag="vextb")
            wrm = big.tile([PB, QT], BF, tag="wrm")     # warmup fodder

            # packed weight views: [p, 4, m]
            wq2_sb = wpk_sb[:, 0:512].rearrange("p (c m) -> p c m", c=4)
            wk2_sb = wpk_sb[:, 512:1024].rearrange("p (c m) -> p c m", c=4)
            wv1_sb = wpk_sb[:, 1024:1280].rearrange("p (c m) -> p c m", c=4)

            dma = nc.sync.dma_start
            dma2 = nc.scalar.dma_start

            # ---- input DMAs, posted on both hwdge queues, ordered by
            # first consumption ----
            dma(wpk_sb[:], wpk_h.ap())
            dma(xtf_sb[:, :, 0:256], xtf_r[:, :, 0:256])
            dma(xtf_sb[:, :, 256:512], xtf_r[:, :, 256:512])
            dma2(mskb_sb[:], mskb_h.ap())
            dma2(msk_sb[:], msk_h.ap())
            for lo, hi in ((0, 512), (512, 1024), (1024, 1536),
                           (1536, 2560), (2560, 3584)):
                dma(xt8_sb[:, :, lo:hi], xt8_r[:, :, lo:hi])

            # PE warmup on a memset tile (no DMA dependency): keeps the PE
            # continuously busy from the preamble until the first input
            # DMA semaphore fires, so the HAM clock gate releases right as
            # real chains start (a single idle gap resets HAM's window and
            # the whole first third runs at half clock).  Many small
            # matmuls give a fine-grained handoff to the real stream.
            # NOTE: few BIG warm matmuls, rotating over all 3 proj psum
            # slots -- many small ones queue so many semaphore increments
            # that the first real chain's psum-WAR wait lands ~2us late.
            nc.vector.memset(wrm[:], 0.03125)
            warm = [pp.tile([PB, QT], F32, tag="proj", name=f"warm{i}")
                    for i in range(3)]
            # stage 1: >=4us continuous so the HAM flip lands mid-warmup,
            # sized to cover the WORST-CASE input-DMA completion semaphore
            # (~14.3us) -- a post-warmup idle gap during the HAM renewal
            # window re-throttles the clock for ~3.4us, which costs far
            # more than the warmup overshoot on lucky runs
            for i in range(12):
                nc.tensor.matmul(warm[i % 3][:], wrm[:, 0:PB], wrm[:],
                                 start=True, stop=True)
            # stage 2: gated on the first input DMA (wpk) -- bridges the
            # run-to-run jitter of the DMA completion semaphores
            for i in range(2):
                nc.tensor.matmul(warm[i % 3][:], wrm[:, 0:PB],
                                 wpk_sb[:, 0:QT], start=True, stop=True)

            # ones columns of V_ext (V blocks overwrite cols 0..63 later)
            nc.vector.memset(vext[:], 1.0)
            nc.vector.memset(vextb[:], 1.0)

            def xseg(c, lo, hi):
                """[128, hi-lo] view of X^T global cols [lo, hi) for weight
                chunk c -- bf16 tile below col 512, fp8 tile above."""
                if hi <= QT:
                    return xtf_sb[:, c, lo:hi]
                assert lo >= QT
                return xt8_sb[:, c, lo - QT:hi - QT]

            def even_blocks_half(c, s4, h):
                """[128, 256] strided view: even 128-col blocks
                {8s4+4h, 8s4+4h+2} of X^T chunk c."""
                seg = xseg(c, 1024 * s4 + 512 * h, 1024 * s4 + 512 * (h + 1))
                return seg.rearrange("p (b two x) -> p b two x",
                                     two=2, x=PB)[:, :, 0, :]

            # Projection chains: specs is a list of
            # ('q', t) | ('qa'/'qb', t) | ('ka'/'kb', s4) | ('v', j).
            # Chains are interleaved per weight chunk so consecutive
            # matmuls alternate PSUM banks (hides PE drain) and short
            # matmuls ride inside long 512-col streams.
            def chains(*specs):
                tiles = [pp.tile([PB, QT], F32, tag="proj",
                                 name=f"{kind}_ps")
                         for kind, idx in specs]
                for c in range(4):
                    for (kind, idx), ps in zip(specs, tiles):
                        if kind == 'q':
                            nc.tensor.matmul(
                                ps[:], wq2_sb[:, c, :],
                                xseg(c, QT * idx, QT * (idx + 1)),
                                start=(c == 0), stop=(c == 3))
                        elif kind in ('qa', 'qb'):
                            h = 0 if kind == 'qa' else 1
                            lo = QT * idx + 256 * h
                            nc.tensor.matmul(
                                ps[:, 0:256], wq2_sb[:, c, :],
                                xseg(c, lo, lo + 256),
                                start=(c == 0), stop=(c == 3))
                        elif kind in ('ka', 'kb'):
                            h = 0 if kind == 'ka' else 1
                            nc.tensor.matmul(
                                ps[:, 0:256], wk2_sb[:, c, :],
                                even_blocks_half(c, idx, h),
                                start=(c == 0), stop=(c == 3))
                        else:
                            nc.tensor.matmul(
                                ps[:, 0:E],
                                xseg(c, 2 * PB * idx, 2 * PB * idx + PB),
                                wv1_sb[:, c, :],
                                start=(c == 0), stop=(c == 3))
                for (kind, idx), ps in zip(specs, tiles):
                    if kind == 'q':
                        nc.vector.tensor_copy(
                            qt2[:, QT * idx:QT * (idx + 1)], ps[:])
                    elif kind in ('qa', 'qb'):
                        h = 0 if kind == 'qa' else 1
                        lo = QT * idx + 256 * h
                        nc.vector.tensor_copy(
                            qt2[:, lo:lo + 256], ps[:, 0:256])
                    elif kind in ('ka', 'kb'):
                        # For the first four tiles the K evacuation rides
                        # on the SCALAR engine (Copy activation): scalar is
                        # idle exactly during those early tile boundaries,
                        # and running it there overlaps the Q evacuation on
                        # Vector -- the tile's first scores (which gate the
                        # exp stream) start sooner.  From tile 4 on the exp
                        # stream is saturated and a scalar copy would delay
                        # exps, so those stay on Vector.
                        h = 0 if kind == 'ka' else 1
                        lo = QT * idx + 256 * h
                        eng = nc.scalar.copy if idx <= 1 else \
                            nc.vector.tensor_copy
                        eng(kt2[:, lo:lo + 256], ps[:, 0:256])
                    elif idx < 2:   # v blocks 0,1 stay bf16
                        nc.vector.tensor_copy(
                            vextb[:, idx, 0:E], ps[:, 0:E])
                    else:
                        nc.vector.tensor_copy(
                            vext[:, idx, 0:E], ps[:, 0:E])

            # ---- main loop over query tiles ----
            pend = []       # deferred PV groups (keeps PE off ACT's tail)
            for t in range(NQT):
                # V blocks (2t, 2t+1) are only read by the deferred PV of
                # tile t, flushed during t+1 -- project them one tile late,
                # riding inside that tile's long chains.  K half-chains and
                # Q chains are scheduled just-in-time against the X DMA
                # strips.
                if t == 0:
                    chains(('qa', 0), ('qb', 0))
                    chains(('ka', 0))
                elif t == 1:
                    chains(('kb', 0), ('q', 1))
                elif t <= 4:
                    # t2-t4: the exp stream still starves at these
                    # boundaries, so the gating K/Q chain runs bare; the V
                    # chains are emitted after this tile's score loop (safe
                    # with the 10-deep PV window: their readers flush later)
                    chains((('ka', 'kb')[t % 2], t // 2), ('q', t))
                elif t < 7:
                    # V chains for blocks (2t-4, 2t-3) ride here: one tile
                    # AFTER their projection inputs land, two tiles BEFORE
                    # their PV reads them -- by now the exp stream has
                    # enough backlog that the extra PE work doesn't starve
                    # ACT.
                    ks = ('ka', 'kb')[t % 2]
                    chains((ks, t // 2), ('q', t), ('v', 2 * t - 4))
                    chains(('v', 2 * t - 3))
                else:
                    # t7 runs bare -- all its V chains were emitted at t6's
                    # post-loop (ACT has ~7 exps of backlog there), so t7's
                    # scores, which the exp-stream tail hangs off, start
                    # ~1.5us earlier
                    chains(('kb', 3), ('q', 7))

                z_ps = zp.tile([E + 1, QT], F32, tag="z", name="z_ps")
                njb = 2 * t + 2
                groups = [list(range(g, min(g + GJ, njb)))
                          for g in range(0, njb, GJ)]
                if t == 7:
                    # diagonal pair first: its extra DVE masking work then
                    # runs early instead of serializing into the kernel
                    # tail.  The final pair is split into two single-block
                    # groups so the tail's serial exp->PV->copy->DMA chain
                    # hangs off a half-width exp.
                    groups = [groups[-1]] + groups[:-2] + [[12], [13]]
                for gi, js in enumerate(groups):
                    s_ps = sp.tile([PB, GJ * QT], F32, tag="s", name="s_ps")
                    for j in js:
                        sl = j - js[0]
                        half = slice(0, 64) if j % 2 == 0 else slice(64, 128)
                        if j == 2 * t + 1:
                            # diagonal-odd block: cols [0,256) fully masked,
                            # compute only the live half (compacted left)
                            nc.tensor.matmul(
                                s_ps[:, QT * sl:QT * sl + 256],
                                kt2[half, PB * j:PB * (j + 1)],
                                qt2[half, QT * t + 256:QT * (t + 1)],
                                start=True, stop=True)
                        else:
                            nc.tensor.matmul(
                                s_ps[:, QT * sl:QT * (sl + 1)],
                                kt2[half, PB * j:PB * (j + 1)],
                                qt2[half, QT * t:QT * (t + 1)],
                                start=True, stop=True)

                    # flush deferred PV matmuls (keep up to 8 in flight so
                    # each tile's last group outlives the NEXT tile's score
                    # loop -- its V blocks are only projected there; drain
                    # hard on the last tile so the kernel tail holds a
                    # single group)
                    lim = 10 if t < 7 else 2
                    while len(pend) >= lim:
                        _flush_pv(nc, pend.pop(0))
                    if t == 7 and gi >= 2:
                        # the endgame is ACT-bound and the PE runs dry for
                        # >3.4us -- the HAM re-throttles the clock and the
                        # FINAL score/PV matmuls (the exp-stream tail) run
                        # at 1.2 GHz.  A dummy matmul per group keeps the
                        # PE busy fraction above the renewal threshold.
                        nc.tensor.matmul(warm[gi % 3][:], wrm[:, 0:PB],
                                         wrm[:], start=True, stop=True)

                    has_ediag = 2 * t in js
                    has_odiag = 2 * t + 1 in js
                    w = QT * len(js) - (256 if has_odiag else 0)
                    bf_pair = js[0] < 2      # first key pair runs in bf16
                    pool = ptbp if bf_pair else ptp
                    pt = pool.tile([PB, GJ * QT], BF if bf_pair else F8,
                                   tag="ptb" if bf_pair else "pt", name="pt")
                    nc.scalar.activation(pt[:, 0:w], s_ps[:, 0:w],
                                         mybir.ActivationFunctionType.Exp,
                                         scale=float(SCALE))
                    mk = mskb_sb if bf_pair else msk_sb
                    if has_ediag:
                        # even-diag: triangular mask in place
                        nc.vector.tensor_mul(
                            pt[:, 0:QT], pt[:, 0:QT], mk[:, 0:QT])
                    if has_odiag:
                        # odd-diag: mask + move compacted live half to its
                        # natural query position, zero the dead left half
                        ob = QT * js.index(2 * t + 1)
                        nc.vector.tensor_mul(
                            pt[:, ob + 256:ob + QT],
                            pt[:, ob:ob + 256], mk[:, 0:256])
                        nc.vector.memset(pt[:, ob:ob + 256], 0.0)
                    pend.append([z_ps, vext, vextb, pt, js, t,
                                 gi == 0, gi == len(groups) - 1])

                # attach Z evacuation of this tile to the last deferred group
                pend[-1].append((zt, zsbp))

                if t in (2, 3, 4):
                    chains(('v', 2 * t - 4), ('v', 2 * t - 3))
                if t == 6:
                    chains(('v', 10), ('v', 11))
                    chains(('v', 12), ('v', 13))
                    chains(('v', 14), ('v', 15))

                # V chains ride AFTER the tile's scores: their results are
                # only read by this tile's deferred PV (flushed >=1 tile
                # later), and emitting them here lets the scores -- which
                # gate the exp stream -- start earlier.


            # tail: flush remaining deferred groups
            for p in pend:
                _flush_pv(nc, p)

    nc.compile()
    return nc


def _flush_pv(nc, pend):
    """Emit the deferred PV matmul group (DoubleRow fp8, or two plain bf16
    matmuls for the first key pair), and Z evacuation if attached."""
    z_ps, vext, vextb, pt, js, t, first, last = pend[:8]
    g = js[0] // 2
    if len(js) == 1:
        vb = vextb[:, js[0], :] if js[0] < 2 else vext[:, js[0], 0:E + 1]
        nc.tensor.matmul(
            z_ps[:], vb, pt[:, 0:QT],
            start=first, stop=last, skip_group_check=True)
    elif g == 0:
        nc.tensor.matmul(
            z_ps[:], vextb[:, 0, :], pt[:, 0:QT],
            start=first, stop=False, skip_group_check=True)
        nc.tensor.matmul(
            z_ps[:], vextb[:, 1, :], pt[:, QT:2 * QT],
            start=False, stop=last, skip_group_check=True)
    else:
        nc.tensor.matmul(
            z_ps[:],
            vext[:, 2 * g:2 * g + 2, 0:E + 1],
            pt[:].rearrange("p (two q) -> p two q", two=2),
            start=first, stop=last,
            perf_mode=DR, skip_group_check=True)
    if len(pend) > 8:
        zt, zsbp = pend[8]
        z_sb = zsbp.tile([E + 1, QT], BF, tag="zsb", name="z_sb")
        if t == 7:
            # tail path: split the evacuation so the first half's DMA
            # posts (on sync) while the second half copies, and the second
            # posts on the now-idle scalar queue
            nc.vector.tensor_copy(z_sb[:, 0:256], z_ps[:, 0:256])
            nc.sync.dma_start(zt[:, QT * t:QT * t + 256], z_sb[:, 0:256])
            nc.vector.tensor_copy(z_sb[:, 256:QT], z_ps[:, 256:QT])
            nc.scalar.dma_start(zt[:, QT * t + 256:QT * (t + 1)],
                                z_sb[:, 256:QT])
        else:
            nc.vector.tensor_copy(z_sb[:], z_ps[:])
            nc.sync.dma_start(zt[:, QT * t:QT * (t + 1)], z_sb[:])


def _get_nc():
    if "nc" not in _CACHE:
        _CACHE["nc"] = _build()
    return _CACHE["nc"]


def _host_inputs(X, Wq, Wk, Wv):
    """Per-core input maps. Core 2b+c: batch b, key parity c; X^T rotated
    left by 128*c columns."""
    w2 = lambda w: np.concatenate([w, w], axis=1).astype(BF16)
    wq2, wk2 = w2(Wq), w2(Wk)
    wv1 = Wv.astype(BF16)
    # pack weights so each partition's chunks are contiguous:
    # [p, (wq2 4x128 | wk2 4x128 | wv1 4x64)]
    pk = lambda w, m: np.ascontiguousarray(
        w.reshape(4, PB, m).transpose(1, 0, 2).reshape(PB, 4 * m))
    wpk = np.concatenate(
        [pk(wq2, PB), pk(wk2, PB), pk(wv1, E)], axis=1)
    # triangular mask: msk[i, u] = 1 if i <= u
    u = np.arange(QT)[None, :]
    i = np.arange(PB)[:, None]
    msk = (i <= u).astype(NPF8)
    mskb = (i <= u).astype(BF16)

    in_maps = []
    for b in range(B):
        xt = np.ascontiguousarray(np.asarray(X[b]).T).astype(BF16)
        for c in (0, 1):
            xtc = xt if c == 0 else np.ascontiguousarray(
                np.roll(xt, -PB * c, axis=1))
            in_maps.append({
                "xtf": np.ascontiguousarray(xtc[:, 0:QT]),
                "xt8": np.ascontiguousarray(xtc[:, QT:]).astype(NPF8),
                "wpk": wpk, "msk": msk, "mskb": mskb,
            })
    return in_maps


def _combine(results):
    Z = np.empty((B, S, E), np.float32)
    for b in range(B):
        za = results[2 * b]["zt"].astype(np.float32)
        zb = np.roll(results[2 * b + 1]["zt"].astype(np.float32),
                     PB, axis=1)     # un-rotate core B's query columns
        # B's wrapped query block (global q < 128) is garbage; A covers it.
        zb[:, 0:PB] = 0.0
        num = za[:E] + zb[:E]
        den = za[E] + zb[E]
        Z[b] = (num / den[None, :]).T
    return Z


def kernel(X, Wq, Wk, Wv, _trace=False, _tmpdir=None):
    from concourse.bass_utils import run_bass_kernel_spmd
    nc = _get_nc()
    in_maps = _host_inputs(X, Wq, Wk, Wv)
    kw = {}
    if _tmpdir is not None:
        kw["tmpdir"] = _tmpdir
    res = run_bass_kernel_spmd(nc, in_maps, core_ids=list(range(NCORES)),
                               trace=_trace, **kw)
    _CACHE["last"] = res
    return _combine(res.results)


# revision 53
# speedup vs baseline: 1.0054x; 1.0054x over previous
"""Causal single-head attention on 8 Trainium2 NeuronCores (Bass/Tile).

Problem: X[4,4096,512] fp32, Wq/Wk/Wv[512,64] fp32.
  Q=XWq, K=XWk, V=XWv ; Z = softmax(mask(QK^T)/8) V    -> [4,4096,64]

Sharding: 2 cores per batch, fully uniform SPMD program.
  - Keys/values are split by PARITY of 128-row key blocks: core A of a pair
    owns even key blocks, core B odd ones.  Each core's X^T input is
    ROTATED left by 128*parity columns by the host, which makes "my key
    blocks" sit at even 128-col positions for BOTH cores -- so one
    instruction stream with static addresses serves both.
  - Each core computes, for every query tile, partial attention over its
    own half of the keys with un-normalized softmax (no max subtraction --
    logits here are ~N(0, 0.2^2) so exp cannot overflow):
        numerator   N_c = sum_k exp(s)*V,   denominator D_c = sum_k exp(s)
    The host combines  Z = (N_A + N_B) / (D_A + D_B)  exactly.  The
    rotation wraps one query block on core B (tile 7); the host simply
    uses A-only partials for those 128 queries (A covers them fully).
  - Denominators come for free as column 64 of V_ext = [V | 1] in the
    P^T @ V_ext matmul.
  - Causality at 128-block granularity is structural (k-block count grows
    with the query tile); diagonal blocks are fixed by multiplying exp(S)
    by a static triangular mask (rotation makes the needed mask content
    identical on both cores).

On-chip dataflow:
  - scores are computed transposed  S^T[k,q] = K^T-block-stationary @ Q^T
    (bf16) so P^T = exp(S^T) feeds the PV matmul with no on-chip
    transpose.  Q^T and K^T are doubled across the partition dim ([W|W]
    weights) so score matmuls (contraction=64) run 2x packed in the PE
    array via row groups (partitions 0-63 / 64-127).
  - P^T and V_ext are stored fp8e4 and the PV matmuls run
    perf_mode=DoubleRow: each matmul contracts a PAIR of 128-key blocks
    (virtual 256-deep PE array) in one 512-column stream -- halving PE
    time on the dominant matmul category.  The FIRST key-block pair of
    each core stays bf16 (plain matmuls): early queries average over few
    keys, so fp8 noise on V does not cancel there (rel err 3e-2 -> 3e-3).
  - V is produced in natural [k,64] layout by making the X^T chunk the
    stationary operand; K projection reads even 128-col blocks of X^T via
    a strided access pattern, split in half-chains scheduled just-in-time
    against the X DMA stream.
  - X^T tokens 512+ ship as fp8e4 (mixed bf16-weight x fp8-X matmuls):
    projection noise only perturbs softmax weights / well-averaged V
    contributions, halving 3.5 MB of the DMA stream at no accuracy cost.
    Tokens 0:512 stay bf16 (they feed the bf16 V blocks).  All weights
    ship as ONE host-packed contiguous tensor (2.5 KB per partition) so
    the first DMA lands fast.  Z ships bf16 (halves the output tail).
  - A two-stage memset-fed warmup matmul block keeps the PE continuously
    busy from the preamble until the first input DMA semaphore fires:
    >=4us of it guarantees the HAM clock-gate release lands mid-warmup
    (one idle gap during the ramp re-throttles the clock to 1.2 GHz for
    ~the whole first third); a short second stage gated on the first
    DMA's semaphore absorbs run-to-run DMA completion jitter.
"""

import numpy as np
import ml_dtypes

import concourse.bacc as bacc
import concourse.bass as bass
import concourse.mybir as mybir
import concourse.tile as tile

B, S, DIN, E = 4, 4096, 512, 64
PB = 128            # partition / key block
QT = 512            # query tile width
NQT = S // QT       # 8 query tiles
NKB = S // PB       # 32 key blocks per batch
HKB = NKB // 2      # 16 packed key blocks per core
SH = S // 2         # 2048 packed keys per core
NCORES = 8
SCALE = 1.0 / np.sqrt(E)
GJ = 2              # k-blocks per exp group (= one DoubleRow PV pair)
VW = 80             # padded V_ext block width (Ko step must be %16==0)

BF16 = ml_dtypes.bfloat16
NPF8 = ml_dtypes.float8_e4m3
BF = mybir.dt.bfloat16
F8 = mybir.dt.float8e4
F32 = mybir.dt.float32
DR = mybir.MatmulPerfMode.DoubleRow

_CACHE = {}


def _build():
    nc = bacc.Bacc("TRN2", target_bir_lowering=False, debug=False,
                   enable_asserts=False, num_devices=NCORES)

    xtf_h = nc.dram_tensor("xtf", [DIN, QT], BF, kind="ExternalInput")
    xt8_h = nc.dram_tensor("xt8", [DIN, S - QT], F8, kind="ExternalInput")
    wpk_h = nc.dram_tensor("wpk", [PB, 1280], BF, kind="ExternalInput")
    msk_h = nc.dram_tensor("msk", [PB, QT], F8, kind="ExternalInput")
    mskb_h = nc.dram_tensor("mskb", [PB, QT], BF, kind="ExternalInput")
    zt_h = nc.dram_tensor("zt", [E + 1, S], BF, kind="ExternalOutput")

    xtf_r = xtf_h.ap().rearrange("(c p) s -> p c s", p=PB)
    xt8_r = xt8_h.ap().rearrange("(c p) s -> p c s", p=PB)
    zt = zt_h.ap()

    with tile.TileContext(nc) as tc:
        with (
            tc.tile_pool(name="big", bufs=1) as big,
            tc.tile_pool(name="pt", bufs=10) as ptp,
            tc.tile_pool(name="ptb", bufs=5) as ptbp,
            tc.tile_pool(name="zsb", bufs=2) as zsbp,
            tc.tile_pool(name="ppsum", bufs=3, space="PSUM") as pp,
            tc.tile_pool(name="spsum", bufs=2, space="PSUM") as sp,
            tc.tile_pool(name="zpsum", bufs=1, space="PSUM") as zp,
        ):
            # ---- persistent SBUF buffers ----
            # X^T tokens 0:512 in bf16 (early-query V/P precision);
            # tokens 512: in fp8 (softmax noise cancels -- halves DMA)
            xtf_sb = big.tile([PB, 4, QT], BF, tag="xtf")
            xt8_sb = big.tile([PB, 4, S - QT], F8, tag="xt8")
            wpk_sb = big.tile([PB, 1280], BF, tag="wpk")
            msk_sb = big.tile([PB, QT], F8, tag="msk")
            mskb_sb = big.tile([PB, QT], BF, tag="mskb")
            qt2 = big.tile([PB, S], BF, tag="qt2")      # doubled Q^T (rot)
            kt2 = big.tile([PB, SH], BF, tag="kt2")     # doubled K^T (packed)
            vext = big.tile([PB, HKB, VW], F8, tag="vext")
            vextb = big.tile([PB, 2, E + 1], BF, tag="vextb")
            wrm = big.tile([PB, QT], BF, tag="wrm")     # warmup fodder

            # packed weight views: [p, 4, m]
            wq2_sb = wpk_sb[:, 0:512].rearrange("p (c m) -> p c m", c=4)
            wk2_sb = wpk_sb[:, 512:1024].rearrange("p (c m) -> p c m", c=4)
            wv1_sb = wpk_sb[:, 1024:1280].rearrange("p (c m) -> p c m", c=4)

            dma = nc.sync.dma_start
            dma2 = nc.scalar.dma_start

            # ---- input DMAs, posted on both hwdge queues, ordered by
            # first consumption ----
            dma(wpk_sb[:], wpk_h.ap())
            dma(xtf_sb[:, :, 0:256], xtf_r[:, :, 0:256])
            dma(xtf_sb[:, :, 256:512], xtf_r[:, :, 256:512])
            dma2(mskb_sb[:], mskb_h.ap())
            dma2(msk_sb[:], msk_h.ap())
            for lo, hi in ((0, 512), (512, 1024), (1024, 1536),
                           (1536, 2560), (2560, 3584)):
                dma(xt8_sb[:, :, lo:hi], xt8_r[:, :, lo:hi])

            # PE warmup on a memset tile (no DMA dependency): keeps the PE
            # continuously busy from the preamble until the first input
            # DMA semaphore fires, so the HAM clock gate releases right as
            # real chains start (a single idle gap resets HAM's window and
            # the whole first third runs at half clock).  Many small
            # matmuls give a fine-grained handoff to the real stream.
            # NOTE: few BIG warm matmuls, rotating over all 3 proj psum
            # slots -- many small ones queue so many semaphore increments
            # that the first real chain's psum-WAR wait lands ~2us late.
            nc.vector.memset(wrm[:], 0.03125)
            warm = [pp.tile([PB, QT], F32, tag="proj", name=f"warm{i}")
                    for i in range(3)]
            # stage 1: >=4us continuous so the HAM flip lands mid-warmup,
            # sized to cover the WORST-CASE input-DMA completion semaphore
            # (~14.3us) -- a post-warmup idle gap during the HAM renewal
            # window re-throttles the clock for ~3.4us, which costs far
            # more than the warmup overshoot on lucky runs
            for i in range(12):
                nc.tensor.matmul(warm[i % 3][:], wrm[:, 0:PB], wrm[:],
                                 start=True, stop=True)


            # ones columns of V_ext (V blocks overwrite cols 0..63 later)
            nc.vector.memset(vext[:], 1.0)
            nc.vector.memset(vextb[:], 1.0)

            def xseg(c, lo, hi):
                """[128, hi-lo] view of X^T global cols [lo, hi) for weight
                chunk c -- bf16 tile below col 512, fp8 tile above."""
                if hi <= QT:
                    return xtf_sb[:, c, lo:hi]
                assert lo >= QT
                return xt8_sb[:, c, lo - QT:hi - QT]

            def even_blocks_half(c, s4, h):
                """[128, 256] strided view: even 128-col blocks
                {8s4+4h, 8s4+4h+2} of X^T chunk c."""
                seg = xseg(c, 1024 * s4 + 512 * h, 1024 * s4 + 512 * (h + 1))
                return seg.rearrange("p (b two x) -> p b two x",
                                     two=2, x=PB)[:, :, 0, :]

            # Projection chains: specs is a list of
            # ('q', t) | ('qa'/'qb', t) | ('ka'/'kb', s4) | ('v', j).
            # Chains are interleaved per weight chunk so consecutive
            # matmuls alternate PSUM banks (hides PE drain) and short
            # matmuls ride inside long 512-col streams.
            def chains(*specs):
                tiles = [pp.tile([PB, QT], F32, tag="proj",
                                 name=f"{kind}_ps")
                         for kind, idx in specs]
                for c in range(4):
                    for (kind, idx), ps in zip(specs, tiles):
                        if kind == 'q':
                            nc.tensor.matmul(
                                ps[:], wq2_sb[:, c, :],
                                xseg(c, QT * idx, QT * (idx + 1)),
                                start=(c == 0), stop=(c == 3))
                        elif kind in ('qa', 'qb'):
                            h = 0 if kind == 'qa' else 1
                            lo = QT * idx + 256 * h
                            nc.tensor.matmul(
                                ps[:, 0:256], wq2_sb[:, c, :],
                                xseg(c, lo, lo + 256),
                                start=(c == 0), stop=(c == 3))
                        elif kind in ('ka', 'kb'):
                            h = 0 if kind == 'ka' else 1
                            nc.tensor.matmul(
                                ps[:, 0:256], wk2_sb[:, c, :],
                                even_blocks_half(c, idx, h),
                                start=(c == 0), stop=(c == 3))
                        else:
                            nc.tensor.matmul(
                                ps[:, 0:E],
                                xseg(c, 2 * PB * idx, 2 * PB * idx + PB),
                                wv1_sb[:, c, :],
                                start=(c == 0), stop=(c == 3))
                for (kind, idx), ps in zip(specs, tiles):
                    if kind == 'q':
                        nc.vector.tensor_copy(
                            qt2[:, QT * idx:QT * (idx + 1)], ps[:])
                    elif kind in ('qa', 'qb'):
                        h = 0 if kind == 'qa' else 1
                        lo = QT * idx + 256 * h
                        nc.vector.tensor_copy(
                            qt2[:, lo:lo + 256], ps[:, 0:256])
                    elif kind in ('ka', 'kb'):
                        # For the first four tiles the K evacuation rides
                        # on the SCALAR engine (Copy activation): scalar is
                        # idle exactly during those early tile boundaries,
                        # and running it there overlaps the Q evacuation on
                        # Vector -- the tile's first scores (which gate the
                        # exp stream) start sooner.  From tile 4 on the exp
                        # stream is saturated and a scalar copy would delay
                        # exps, so those stay on Vector.
                        h = 0 if kind == 'ka' else 1
                        lo = QT * idx + 256 * h
                        eng = nc.scalar.copy if idx <= 1 else \
                            nc.vector.tensor_copy
                        eng(kt2[:, lo:lo + 256], ps[:, 0:256])
                    elif idx < 2:   # v blocks 0,1 stay bf16
                        nc.vector.tensor_copy(
                            vextb[:, idx, 0:E], ps[:, 0:E])
                    else:
                        nc.vector.tensor_copy(
                            vext[:, idx, 0:E], ps[:, 0:E])

            # ---- main loop over query tiles ----
            pend = []       # deferred PV groups (keeps PE off ACT's tail)
            for t in range(NQT):
                # V blocks (2t, 2t+1) are only read by the deferred PV of
                # tile t, flushed during t+1 -- project them one tile late,
                # riding inside that tile's long chains.  K half-chains and
                # Q chains are scheduled just-in-time against the X DMA
                # strips.
                if t == 0:
                    chains(('qa', 0), ('qb', 0))
                    chains(('ka', 0))
                elif t == 1:
                    chains(('kb', 0), ('q', 1))
                elif t <= 4:
                    # t2-t4: the exp stream still starves at these
                    # boundaries, so the gating K/Q chain runs bare; the V
                    # chains are emitted after this tile's score loop (safe
                    # with the 10-deep PV window: their readers flush later)
                    chains((('ka', 'kb')[t % 2], t // 2), ('q', t))
                elif t < 7:
                    # V chains for blocks (2t-4, 2t-3) ride here: one tile
                    # AFTER their projection inputs land, two tiles BEFORE
                    # their PV reads them -- by now the exp stream has
                    # enough backlog that the extra PE work doesn't starve
                    # ACT.
                    ks = ('ka', 'kb')[t % 2]
                    chains((ks, t // 2), ('q', t), ('v', 2 * t - 4))
                    chains(('v', 2 * t - 3))
                else:
                    # t7 runs bare -- all its V chains were emitted at t6's
                    # post-loop (ACT has ~7 exps of backlog there), so t7's
                    # scores, which the exp-stream tail hangs off, start
                    # ~1.5us earlier
                    chains(('kb', 3), ('q', 7))

                z_ps = zp.tile([E + 1, QT], F32, tag="z", name="z_ps")
                njb = 2 * t + 2
                groups = [list(range(g, min(g + GJ, njb)))
                          for g in range(0, njb, GJ)]
                if t == 7:
                    # diagonal pair first: its extra DVE masking work then
                    # runs early instead of serializing into the kernel
                    # tail.  The final pair is split into two single-block
                    # groups so the tail's serial exp->PV->copy->DMA chain
                    # hangs off a half-width exp.
                    groups = [groups[-1]] + groups[:-2] + [[12], [13]]
                for gi, js in enumerate(groups):
                    s_ps = sp.tile([PB, GJ * QT], F32, tag="s", name="s_ps")
                    for j in js:
                        sl = j - js[0]
                        half = slice(0, 64) if j % 2 == 0 else slice(64, 128)
                        if j == 2 * t + 1:
                            # diagonal-odd block: cols [0,256) fully masked,
                            # compute only the live half (compacted left)
                            nc.tensor.matmul(
                                s_ps[:, QT * sl:QT * sl + 256],
                                kt2[half, PB * j:PB * (j + 1)],
                                qt2[half, QT * t + 256:QT * (t + 1)],
                                start=True, stop=True)
                        else:
                            nc.tensor.matmul(
                                s_ps[:, QT * sl:QT * (sl + 1)],
                                kt2[half, PB * j:PB * (j + 1)],
                                qt2[half, QT * t:QT * (t + 1)],
                                start=True, stop=True)

                    # flush deferred PV matmuls (keep up to 8 in flight so
                    # each tile's last group outlives the NEXT tile's score
                    # loop -- its V blocks are only projected there; drain
                    # hard on the last tile so the kernel tail holds a
                    # single group)
                    lim = 10 if t < 7 else 2
                    while len(pend) >= lim:
                        _flush_pv(nc, pend.pop(0))
                    if t == 7 and gi >= 2:
                        # the endgame is ACT-bound and the PE runs dry for
                        # >3.4us -- the HAM re-throttles the clock and the
                        # FINAL score/PV matmuls (the exp-stream tail) run
                        # at 1.2 GHz.  A dummy matmul per group keeps the
                        # PE busy fraction above the renewal threshold.
                        nc.tensor.matmul(warm[gi % 3][:], wrm[:, 0:PB],
                                         wrm[:], start=True, stop=True)

                    has_ediag = 2 * t in js
                    has_odiag = 2 * t + 1 in js
                    w = QT * len(js) - (256 if has_odiag else 0)
                    bf_pair = js[0] < 2      # first key pair runs in bf16
                    pool = ptbp if bf_pair else ptp
                    pt = pool.tile([PB, GJ * QT], BF if bf_pair else F8,
                                   tag="ptb" if bf_pair else "pt", name="pt")
                    nc.scalar.activation(pt[:, 0:w], s_ps[:, 0:w],
                                         mybir.ActivationFunctionType.Exp,
                                         scale=float(SCALE))
                    mk = mskb_sb if bf_pair else msk_sb
                    if has_ediag:
                        # even-diag: triangular mask in place
                        nc.vector.tensor_mul(
                            pt[:, 0:QT], pt[:, 0:QT], mk[:, 0:QT])
                    if has_odiag:
                        # odd-diag: mask + move compacted live half to its
                        # natural query position, zero the dead left half
                        ob = QT * js.index(2 * t + 1)
                        nc.vector.tensor_mul(
                            pt[:, ob + 256:ob + QT],
                            pt[:, ob:ob + 256], mk[:, 0:256])
                        nc.vector.memset(pt[:, ob:ob + 256], 0.0)
                    pend.append([z_ps, vext, vextb, pt, js, t,
                                 gi == 0, gi == len(groups) - 1])

                # attach Z evacuation of this tile to the last deferred group
                pend[-1].append((zt, zsbp))

                if t in (2, 3, 4):
                    chains(('v', 2 * t - 4), ('v', 2 * t - 3))
                if t == 6:
                    chains(('v', 10), ('v', 11))
                    chains(('v', 12), ('v', 13))
                    chains(('v', 14), ('v', 15))

                # V chains ride AFTER the tile's scores: their results are
                # only read by this tile's deferred PV (flushed >=1 tile
                # later), and emitting them here lets the scores -- which
                # gate the exp stream -- start earlier.


            # tail: flush remaining deferred groups
            for p in pend:
                _flush_pv(nc, p)

    nc.compile()
    return nc


def _flush_pv(nc, pend):
    """Emit the deferred PV matmul group (DoubleRow fp8, or two plain bf16
    matmuls for the first key pair), and Z evacuation if attached."""
    z_ps, vext, vextb, pt, js, t, first, last = pend[:8]
    g = js[0] // 2
    if len(js) == 1:
        vb = vextb[:, js[0], :] if js[0] < 2 else vext[:, js[0], 0:E + 1]
        nc.tensor.matmul(
            z_ps[:], vb, pt[:, 0:QT],
            start=first, stop=last, skip_group_check=True)
    elif g == 0:
        nc.tensor.matmul(
            z_ps[:], vextb[:, 0, :], pt[:, 0:QT],
            start=first, stop=False, skip_group_check=True)
        nc.tensor.matmul(
            z_ps[:], vextb[:, 1, :], pt[:, QT:2 * QT],
            start=False, stop=last, skip_group_check=True)
    else:
        nc.tensor.matmul(
            z_ps[:],
            vext[:, 2 * g:2 * g + 2, 0:E + 1],
            pt[:].rearrange("p (two q) -> p two q", two=2),
            start=first, stop=last,
            perf_mode=DR, skip_group_check=True)
    if len(pend) > 8:
        zt, zsbp = pend[8]
        z_sb = zsbp.tile([E + 1, QT], BF, tag="zsb", name="z_sb")
        if t == 7:
            # tail path: split the evacuation so the first half's DMA
            # posts (on sync) while the second half copies, and the second
            # posts on the now-idle scalar queue
            nc.vector.tensor_copy(z_sb[:, 0:256], z_ps[:, 0:256])
            nc.sync.dma_start(zt[:, QT * t:QT * t + 256], z_sb[:, 0:256])
            nc.vector.tensor_copy(z_sb[:, 256:QT], z_ps[:, 256:QT])
            nc.scalar.dma_start(zt[:, QT * t + 256:QT * (t + 1)],
                                z_sb[:, 256:QT])
        else:
            nc.vector.tensor_copy(z_sb[:], z_ps[:])
            nc.sync.dma_start(zt[:, QT * t:QT * (t + 1)], z_sb[:])


def _get_nc():
    if "nc" not in _CACHE:
        _CACHE["nc"] = _build()
    return _CACHE["nc"]


def _host_inputs(X, Wq, Wk, Wv):
    """Per-core input maps. Core 2b+c: batch b, key parity c; X^T rotated
    left by 128*c columns."""
    w2 = lambda w: np.concatenate([w, w], axis=1).astype(BF16)
    wq2, wk2 = w2(Wq), w2(Wk)
    wv1 = Wv.astype(BF16)
    # pack weights so each partition's chunks are contiguous:
    # [p, (wq2 4x128 | wk2 4x128 | wv1 4x64)]
    pk = lambda w, m: np.ascontiguousarray(
        w.reshape(4, PB, m).transpose(1, 0, 2).reshape(PB, 4 * m))
    wpk = np.concatenate(
        [pk(wq2, PB), pk(wk2, PB), pk(wv1, E)], axis=1)
    # triangular mask: msk[i, u] = 1 if i <= u
    u = np.arange(QT)[None, :]
    i = np.arange(PB)[:, None]
    msk = (i <= u).astype(NPF8)
    mskb = (i <= u).astype(BF16)

    in_maps = []
    for b in range(B):
        xt = np.ascontiguousarray(np.asarray(X[b]).T).astype(BF16)
        for c in (0, 1):
            xtc = xt if c == 0 else np.ascontiguousarray(
                np.roll(xt, -PB * c, axis=1))
            in_maps.append({
                "xtf": np.ascontiguousarray(xtc[:, 0:QT]),
                "xt8": np.ascontiguousarray(xtc[:, QT:]).astype(NPF8),
                "wpk": wpk, "msk": msk, "mskb": mskb,
            })
    return in_maps


def _combine(results):
    Z = np.empty((B, S, E), np.float32)
    for b in range(B):
        za = results[2 * b]["zt"].astype(np.float32)
        zb = np.roll(results[2 * b + 1]["zt"].astype(np.float32),
                     PB, axis=1)     # un-rotate core B's query columns
        # B's wrapped query block (global q < 128) is garbage; A covers it.
        zb[:, 0:PB] = 0.0
        num = za[:E] + zb[:E]
        den = za[E] + zb[E]
        Z[b] = (num / den[None, :]).T
    return Z


def kernel(X, Wq, Wk, Wv, _trace=False, _tmpdir=None):
    from concourse.bass_utils import run_bass_kernel_spmd
    nc = _get_nc()
    in_maps = _host_inputs(X, Wq, Wk, Wv)
    kw = {}
    if _tmpdir is not None:
        kw["tmpdir"] = _tmpdir
    res = run_bass_kernel_spmd(nc, in_maps, core_ids=list(range(NCORES)),
                               trace=_trace, **kw)
    _CACHE["last"] = res
    return _combine(res.results)
